# revision 32
# baseline (speedup 1.0000x reference)
"""Multi-headed attention kernel for Trainium2, SPMD across 8 NeuronCores.

Problem: B=4, S=2048, D_MODEL=1024, HEADS=16, D_HEAD=64 (fp32).

Sharding: batch across cores (4 batches x 2 cores each); within a batch pair,
heads are split 8+8 (tensor parallel). Each core computes, for its
(batch b, head half hh):
    Q^T = (Wq_s)^T X_q^T + bq   [512, 2048]   (hd-major, bf16)
    K^T = (Wk_s)^T X_k^T + bk   [512, 2048]   (bf16)
    V'' = X_v Wv_s + bv         [2048, 8*65]  (bf16; per-head 65th col = 8.0)
    per head-pair g, sq-chunk c (512 cols), k-tile t:
        L^T[hl] = K_h Q_h^T         (bf16 matmuls into a [128,1024] PSUM tile)
        P = exp(L^T)                (one 1024-wide ACT op, bf16 out)
        P *= mask01                 (DVE bf16 multiply, 2x mode; the mask tile
                                     is free-dim broadcast across both heads)
        AV^T[65,512] += V''_h^T P   (bf16 matmuls; row 64 = 8*rowsum)
    normalize: attn^T = AV^T[0:64] * recip(AV^T[64]) (reciprocal_approx_fast
               on a [64,8] DRAM-bounce reshape; broadcast + final multiply on
               the otherwise-idle GPSIMD so the DVE FIFO never waits on the
               reciprocal chain)
    out^T = Wo_s^T attn^T           [1024, 2048]  (bf16, partial over heads)
Host: out[b] = (outT_core0 + outT_core1).T + bo.

DMA economics: weights and x inputs are pre-staged on the host into the
SBUF-native [128, ...] partition-major layout, so each tensor loads with a
single DMA of long contiguous lines (128 descriptors) instead of thousands of
1KB descriptors — the load phase is bandwidth-bound, not descriptor-bound.

Pipelining: the inner t-loop is software-pipelined across block boundaries
(QK for t+1 — including the first iteration of the next block — is emitted
between exp(t) and the AV matmuls), and projection work (K strips after i=0,
Q for chunk c+1, output projection for chunk c-1) is injected as 2-matmul
pieces between attention iterations, so the ACT engine (the exp pacer) never
idles after the short V/K/Q0 startup.

Masking is a multiply by a 0/1 bf16 mask after exp: exp of the raw logit is
finite, and multiplying by 0 reproduces the reference's -1e8 bias + softmax
exactly (masked entries contribute 0 to both numerator and rowsum).
"""
import heapq
import numpy as np
import ml_dtypes
from contextlib import ExitStack

import concourse.bass as bass
import concourse.tile as tile
from concourse import bacc, mybir
from concourse.bass_utils import run_bass_kernel_spmd

F32 = mybir.dt.float32
BF16 = mybir.dt.bfloat16

B, S, D, H, DH = 4, 2048, 1024, 16, 64
HPC = 8           # heads per core
HD = HPC * DH     # 512 head-dims per core
NCORES = 8
ET = D // 128     # 8 e-tiles (d_model contraction tiles)
ST = 16           # s tiles
CH = 4            # sq chunks of 512
ADD = mybir.AluOpType.add

last_results = None


def _emit(ctx: ExitStack, tc: tile.TileContext, io: dict):
    nc = tc.nc
    xq_s = io["xq_s"].rearrange("p (e s) -> p e s", e=ET)
    xk_s = io["xk_s"].rearrange("p (e s) -> p e s", e=ET)
    xv_s = io["xv_s"].rearrange("p (e s) -> p e s", e=ET)
    m01_s = io["m01_s"].rearrange("p (t s) -> p t s", t=ST)
    outT = io["outT"]

    const = ctx.enter_context(tc.tile_pool(name="const", bufs=1))
    bigA = ctx.enter_context(tc.tile_pool(name="bigA", bufs=1))
    mkp = ctx.enter_context(tc.tile_pool(name="mkp", bufs=3))
    ppool = ctx.enter_context(tc.tile_pool(name="ppool", bufs=3))
    rpool = ctx.enter_context(tc.tile_pool(name="rpool", bufs=2))
    rdram = ctx.enter_context(tc.tile_pool(name="rdram", bufs=3, space="DRAM"))
    ostg = ctx.enter_context(tc.tile_pool(name="ostg", bufs=2))
    wpool = ctx.enter_context(tc.tile_pool(name="wpool", bufs=1))
    xpool = ctx.enter_context(tc.tile_pool(name="xpool", bufs=1))
    # PSUM: qk 2x2 banks + av 2 + proj 2 = 8
    pqk = ctx.enter_context(tc.tile_pool(name="pqk", bufs=2, space="PSUM"))
    pav = ctx.enter_context(tc.tile_pool(name="pav", bufs=2, space="PSUM"))
    pproj = ctx.enter_context(tc.tile_pool(name="pproj", bufs=2, space="PSUM"))

    # ---- constants ----
    bqh_sb = const.tile([128, 4], F32)
    nc.sync.dma_start(bqh_sb, io["bqh"])
    bkh_sb = const.tile([128, 4], F32)
    nc.sync.dma_start(bkh_sb, io["bkh"])
    bv_bc = const.tile([128, HD], BF16)
    nc.gpsimd.dma_start(bv_bc, io["bvh"].partition_broadcast(128))
    scratch = const.tile([1, 2], F32)

    # ---- persistent tensors ----
    qT = bigA.tile([128, 4, S], BF16)       # [hd-part, g, sq]
    kT = bigA.tile([128, 4, S], BF16)
    vs = bigA.tile([128, ST, HPC * 65], BF16)
    att = bigA.tile([128, 4, S], BF16)      # attn^T [hd, sq]

    # ---- weights (single-DMA, host-staged partition-major) ----
    wq_sb = wpool.tile([128, ET, HD], BF16)
    wk_sb = wpool.tile([128, ET, HD], BF16)
    wv_sb = wpool.tile([128, ET, HD], BF16)
    wo_sb = wpool.tile([128, 4, D], BF16)

    # xk fully resident (one 4MB DMA); xv/xq stream through small rings
    xk_sb = xpool.tile([128, ET, S], BF16)

    mkh = [None] * (2 * CH)   # mask half-tiles: half 2c+j = chunk c, t 8j..8j+7
    xqs, xvs = {}, {}

    def load_mask_half(h):
        mt = mkp.tile([128, 8, 512], BF16, tag="mk", name=f"mk{h}")
        cc, j = h // 2, h % 2
        nc.sync.dma_start(
            mt, m01_s[:, 8 * j:8 * j + 8, cc * 512:(cc + 1) * 512])
        mkh[h] = mt

    def load_xv(cc):
        xt = xpool.tile([128, ET, 512], BF16, tag="xv",
                        name=f"xv{cc}", bufs=2)
        nc.sync.dma_start(xt, xv_s[:, :, cc * 512:(cc + 1) * 512])
        xvs[cc] = xt

    def load_xq(cc):
        # per-e tiles in a 16-deep ring: two chunks live, and the reused
        # slot's readers (Qproj of chunk cc-2) are already emitted
        for e in range(ET):
            xt = xpool.tile([128, 512], BF16, tag="xq",
                            name=f"xq_{e}_{cc}", bufs=16)
            nc.sync.dma_start(xt, xq_s[:, e, cc * 512:(cc + 1) * 512])
            xqs[(e, cc)] = xt

    def qk_evac(ps, bias_sb, out_sb, i, cc):
        nc.vector.tensor_scalar(
            out=out_sb[:, i, cc * 512:(cc + 1) * 512], in0=ps,
            scalar1=bias_sb[:, i:i + 1], scalar2=None, op0=ADD)

    def qk_proj_pieces(w_sb, x_of, bias_sb, out_sb, i, cc):
        """One [128,512] strip of a Q/K projection as four 2-matmul pieces.
        The PSUM tile is allocated inside the first piece (at pop time), so
        the pool-ring WAR order always matches the execution order."""
        ps = [None]
        pieces = []
        for e0 in range(0, ET, 2):
            def piece(e0=e0):
                if e0 == 0:
                    ps[0] = pproj.tile([128, 512], F32, tag="po",
                                       name="ps_qk")
                for e in (e0, e0 + 1):
                    nc.tensor.matmul(
                        ps[0], w_sb[:, e, i * 128:(i + 1) * 128], x_of(e, cc),
                        start=(e == 0), stop=(e == ET - 1))
                if e0 == ET - 2:
                    qk_evac(ps[0], bias_sb, out_sb, i, cc)
            pieces.append(piece)
        return pieces

    def out_proj_pieces(dm, cc):
        po = [None]

        def piece_a():
            po[0] = pproj.tile([128, 512], F32, tag="po", name="ps_o")
            for j in (0, 1):
                nc.tensor.matmul(
                    po[0], wo_sb[:, j, dm * 128:(dm + 1) * 128],
                    att[:, j, cc * 512:(cc + 1) * 512],
                    start=(j == 0), stop=False)

        def piece_b():
            for j in (2, 3):
                nc.tensor.matmul(
                    po[0], wo_sb[:, j, dm * 128:(dm + 1) * 128],
                    att[:, j, cc * 512:(cc + 1) * 512],
                    start=False, stop=(j == 3))
            og = ostg.tile([128, 512], F32, tag="og", name="og")
            nc.vector.tensor_copy(og, po[0])
            nc.sync.dma_start(
                outT[dm * 128:(dm + 1) * 128, cc * 512:(cc + 1) * 512], og)
        return [piece_a, piece_b]

    def emit_qk(cc, g, t):
        qk = pqk.tile([128, 1024], F32, tag="qk", name="qk")
        q0 = cc * 512
        for hl in range(2):
            r0 = hl * 64
            nc.tensor.matmul(
                qk[:, hl * 512:(hl + 1) * 512],
                kT[r0:r0 + 64, g, t * 128:(t + 1) * 128],
                qT[r0:r0 + 64, g, q0:q0 + 512],
                start=True, stop=True,
                tile_position=(r0, 0))
        return qk

    # deferred work: heap of (deadline_serial, seq, piece). Force-popped by
    # deadline (so strip writes are always emitted before their readers);
    # slack-filled otherwise in deadline order.
    deferred = []
    defseq = [0]

    def push_def(deadline, pieces):
        for pc in pieces:
            heapq.heappush(deferred, (deadline, defseq[0], pc))
            defseq[0] += 1

    def pop_def(serial, rate):
        n = 0
        while deferred and (deferred[0][0] <= serial + 2 or n < rate):
            heapq.heappop(deferred)[2]()
            n += 1

    def norm_pieces(cc, g, avs):
        """Normalize + evacuate one block's AV pair; runs as deferred work
        inside a later block so the boundary DVE queue stays short. The
        final broadcast-multiply runs on GPSIMD."""
        q0 = cc * 512
        out = []
        for hl in range(2):
            def piece(hl=hl):
                r0 = hl * 64
                avc = rpool.tile([65, 512], F32, tag="avc", name="avc")
                nc.vector.tensor_copy(avc, avs[hl])
                rd = rdram.tile([1, 512], F32, tag="rd", name="rd")
                nc.sync.dma_start(rd, avc[64:65, :])
                rsq = rpool.tile([64, 8], F32, tag="rsq", name="rsq")
                nc.sync.dma_start(
                    rsq, rd.rearrange("one (p j) -> (one p) j", j=8))
                rsr = rpool.tile([64, 8], F32, tag="rsr", name="rsr")
                nc.vector.reciprocal_approx_fast(rsr, rsq)
                rd2 = rdram.tile([1, 512], F32, tag="rd2", name="rd2")
                nc.sync.dma_start(
                    rd2.rearrange("one (p j) -> (one p) j", j=8), rsr)
                rbc = rpool.tile([64, 512], F32, tag="rbc", name="rbc")
                nc.gpsimd.dma_start(rbc, rd2.partition_broadcast(64))
                nc.gpsimd.tensor_mul(
                    att[r0:r0 + 64, g, q0:q0 + 512], avc[0:64, :], rbc)
            out.append(piece)
        return out

    pending_qk = [None]

    def attention_block(cc, g, bi, nxt, hooks):
        base = bi * ST
        avs = []
        for hl in range(2):
            av = pav.tile([65, 512], F32, tag="av", name=f"av{hl}")
            avs.append(av)
        qk_cur = pending_qk[0] if pending_qk[0] is not None else \
            emit_qk(cc, g, 0)
        pending_qk[0] = None
        for t in range(ST):
            if t in hooks:
                hooks[t]()
            p = ppool.tile([128, 1024], BF16, tag="p", name="p")
            nc.scalar.activation(p, qk_cur,
                                 mybir.ActivationFunctionType.Exp)
            # QK for t+1 right behind exp in the PE FIFO so the next exp is
            # never gated by deferred pieces
            if t + 1 < ST:
                qk_nxt = emit_qk(cc, g, t + 1)
            elif nxt is not None:
                qk_nxt = emit_qk(nxt[0], nxt[1], 0)
                pending_qk[0] = qk_nxt
            else:
                qk_nxt = None
            mt = mkh[2 * cc + t // 8]
            m_b = mt[:, t % 8:t % 8 + 1, :].broadcast_to((128, 2, 512))
            p2 = p.rearrange("p (hl q) -> p hl q", q=512)
            nc.vector.tensor_mul(p2, p2, m_b)
            # deferred pieces: after mask (mask leads the DVE queue), before
            # AV (they fill the PE while ACT runs)
            pop_def(base + t, 2 if len(deferred) > 24 else (t % 2))
            for hl in range(2):
                h = 2 * g + hl
                nc.tensor.matmul(
                    avs[hl], vs[:, t, h * 65:(h + 1) * 65],
                    p[:, hl * 512:(hl + 1) * 512],
                    start=(t == 0), stop=(t == ST - 1))
            qk_cur = qk_nxt
        # this block's normalization runs inside the next blocks; it must be
        # emitted before block bi+2 allocates its AV tiles (ring of 2)
        push_def((bi + 2) * ST - 2, norm_pieces(cc, g, avs))

    def v_proj_pieces(s):
        ps = [None]
        pieces = []
        for e0 in range(0, ET, 2):
            def piece(e0=e0):
                if e0 == 0:
                    ps[0] = pproj.tile([128, 512], F32, tag="po",
                                       name="ps_v")
                for e in (e0, e0 + 1):
                    nc.tensor.matmul(
                        ps[0],
                        xvs[s // 4][:, e, (s % 4) * 128:(s % 4 + 1) * 128],
                        wv_sb[:, e, :], start=(e == 0), stop=(e == ET - 1))
                if e0 == ET - 2:
                    nc.vector.tensor_add(
                        out=vs[:, s, :].rearrange(
                            "p (h dd) -> p h dd", dd=65)[:, :, 0:64],
                        in0=ps[0].rearrange("p (h d) -> p h d", d=64),
                        in1=bv_bc.rearrange("p (h d) -> p h d", d=64))
            pieces.append(piece)
        return pieces

    def xk_of(e, cc):
        return xk_sb[:, e, cc * 512:(cc + 1) * 512]

    def xq_of(e, cc):
        return xqs[(e, cc)]

    # ===== emission (DMAs in consumption order) =====
    nc.sync.dma_start(wv_sb, io["wv_s"].rearrange("p (e h) -> p e h", e=ET))
    load_xv(0)
    nc.vector.memset(scratch, 0.0)
    nc.scalar.activation(scratch[:, 0:1], scratch[:, 1:2],
                         mybir.ActivationFunctionType.Exp)
    ones_view = vs.rearrange("p s (h dd) -> p s h dd", dd=65)[:, :, :, 64:65]
    nc.vector.memset(ones_view, 8.0)
    nc.sync.dma_start(wk_sb, io["wk_s"].rearrange("p (e h) -> p e h", e=ET))
    nc.sync.dma_start(xk_sb, xk_s)
    nc.sync.dma_start(wq_sb, io["wq_s"].rearrange("p (e h) -> p e h", e=ET))
    load_xq(0)
    load_mask_half(0)
    load_mask_half(1)
    load_xv(1)
    nc.sync.dma_start(wo_sb, io["wo_s"].rearrange("p (j f) -> p j f", j=4))

    # inline minimum: V s0..3, K strip (i=0, cc=0), Q strip (c0, i=0);
    # everything else is deadline-deferred into the attention blocks
    for s in range(4):
        for piece in v_proj_pieces(s):
            piece()
    for piece in qk_proj_pieces(wk_sb, xk_of, bkh_sb, kT, 0, 0):
        piece()
    for piece in qk_proj_pieces(wq_sb, xq_of, bqh_sb, qT, 0, 0):
        piece()
    load_mask_half(2)

    for s in range(4, ST):
        push_def(s, v_proj_pieces(s))
    for cc in range(1, CH):
        push_def(4 * cc - 2, qk_proj_pieces(wk_sb, xk_of, bkh_sb, kT, 0, cc))
    for i in range(1, 4):
        push_def(ST * i - 2, qk_proj_pieces(wq_sb, xq_of, bqh_sb, qT, i, 0))
    for i in range(1, 4):
        for cc in range(CH):
            push_def(ST * i + 4 * cc - 2,
                     qk_proj_pieces(wk_sb, xk_of, bkh_sb, kT, i, cc))

    FAR = 10 ** 9
    blocks = [(c, g) for c in range(CH) for g in range(4)]
    for bi, (c, g) in enumerate(blocks):
        hooks = {}
        if g == 0 and c + 1 < CH:
            load_xq(c + 1)
        if c == 0 and g == 0:
            hooks[0] = lambda: load_xv(2)
            hooks[6] = lambda: load_xv(3)
        if c == 0 and g == 3:
            for i in range(4):
                push_def(ST * (4 + i) - 2,
                         qk_proj_pieces(wq_sb, xq_of, bqh_sb, qT, i, 1))
        # mask half-tile ring (3 slots): the reused slot's readers are always
        # fully emitted at these points
        if c >= 1 and g == 0:
            load_mask_half(2 * c + 1)
        if c >= 1 and g == 1 and c + 1 < CH:
            load_mask_half(2 * (c + 1))
        if c >= 1:
            if g == 0 and c + 1 < CH:
                for i in range(4):
                    push_def(ST * (4 * (c + 1) + i) - 2,
                             qk_proj_pieces(wq_sb, xq_of, bqh_sb, qT,
                                            i, c + 1))
            push_def(FAR, out_proj_pieces(2 * g, c - 1))
            push_def(FAR, out_proj_pieces(2 * g + 1, c - 1))
        nxt = blocks[bi + 1] if bi + 1 < len(blocks) else None
        attention_block(c, g, bi, nxt, hooks)
    while deferred:
        heapq.heappop(deferred)[2]()
    for dm in range(ET):
        for piece in out_proj_pieces(dm, CH - 1):
            piece()


def build_nc():
    nc = bacc.Bacc("TRN2", target_bir_lowering=False, debug=False,
                   num_devices=NCORES)
    io = {}
    for name, shape, dt_, kind in [
        ("xq_s", [128, ET * S], BF16, "ExternalInput"),
        ("xk_s", [128, ET * S], BF16, "ExternalInput"),
        ("xv_s", [128, ET * S], BF16, "ExternalInput"),
        ("wq_s", [128, ET * HD], BF16, "ExternalInput"),
        ("wk_s", [128, ET * HD], BF16, "ExternalInput"),
        ("wv_s", [128, ET * HD], BF16, "ExternalInput"),
        ("wo_s", [128, 4 * D], BF16, "ExternalInput"),
        ("bqh", [128, 4], F32, "ExternalInput"),
        ("bkh", [128, 4], F32, "ExternalInput"),
        ("bvh", [HD], BF16, "ExternalInput"),
        ("m01_s", [128, ST * S], BF16, "ExternalInput"),
        ("outT", [D, S], F32, "ExternalOutput"),
    ]:
        io[name] = nc.dram_tensor(name, shape, dt_, kind=kind).ap()
    with tile.TileContext(nc) as tc:
        with ExitStack() as ctx:
            _emit(ctx, tc, io)
    nc.compile()
    return nc


def _stage_pmajor(a, tiles, rows=128):
    """[tiles*rows, X] -> [rows, tiles*X] partition-major staging."""
    t, x = tiles, a.shape[1]
    return np.ascontiguousarray(
        a.reshape(t, rows, x).transpose(1, 0, 2).reshape(rows, t * x))


def make_in_maps(query, key_, value, mask, Wq, bq, Wk, bk, Wv, bv, Wo, bo):
    bf = ml_dtypes.bfloat16
    in_maps = []
    for c in range(NCORES):
        b, hh = c // 2, c % 2
        h0 = hh * HPC
        m01T = (~mask[b]).T.astype(np.float32)          # [sk, sq]
        in_maps.append({
            "xq_s": _stage_pmajor(query[b].T.astype(bf), ET),
            "xk_s": _stage_pmajor(key_[b].T.astype(bf), ET),
            "xv_s": _stage_pmajor(value[b].T.astype(bf), ET),
            "wq_s": _stage_pmajor(
                Wq[:, h0:h0 + HPC, :].reshape(D, HD).astype(bf), ET),
            "wk_s": _stage_pmajor(
                Wk[:, h0:h0 + HPC, :].reshape(D, HD).astype(bf), ET),
            "wv_s": _stage_pmajor(
                Wv[:, h0:h0 + HPC, :].reshape(D, HD).astype(bf), ET),
            "wo_s": _stage_pmajor(
                Wo[h0:h0 + HPC].reshape(HD, D).astype(bf), 4),
            "bqh": np.ascontiguousarray(bq[h0:h0 + HPC].reshape(4, 128).T),
            "bkh": np.ascontiguousarray(bk[h0:h0 + HPC].reshape(4, 128).T),
            "bvh": np.ascontiguousarray(
                bv[h0:h0 + HPC].reshape(HD)).astype(bf),
            "m01_s": _stage_pmajor(m01T.astype(bf), ST),
        })
    return in_maps


_nc_cache = None


def kernel(query, key_, value, mask, Wq, bq, Wk, bk, Wv, bv, Wo, bo):
    global last_results, _nc_cache
    query = np.asarray(query, dtype=np.float32)
    key_ = np.asarray(key_, dtype=np.float32)
    value = np.asarray(value, dtype=np.float32)
    mask = np.asarray(mask, dtype=bool)
    Wq, bq = np.asarray(Wq, np.float32), np.asarray(bq, np.float32)
    Wk, bk = np.asarray(Wk, np.float32), np.asarray(bk, np.float32)
    Wv, bv = np.asarray(Wv, np.float32), np.asarray(bv, np.float32)
    Wo, bo = np.asarray(Wo, np.float32), np.asarray(bo, np.float32)

    if _nc_cache is None:
        _nc_cache = build_nc()
    in_maps = make_in_maps(query, key_, value, mask, Wq, bq, Wk, bk,
                           Wv, bv, Wo, bo)
    res = run_bass_kernel_spmd(_nc_cache, in_maps, core_ids=list(range(NCORES)))
    last_results = res
    out = np.empty((B, S, D), dtype=np.float32)
    for b in range(B):
        acc = res.results[2 * b]["outT"].astype(np.float32) + \
            res.results[2 * b + 1]["outT"].astype(np.float32)
        out[b] = acc.T + bo[None, :]
    return out
